# revision 5
# baseline (speedup 1.0000x reference)
"""Causal self-attention (B=2, T=2048, C=1024, H=16) on 8 Trainium2 NeuronCores.

Sharding: tensor-parallel over heads — each core owns 2 heads for BOTH
batches.  The 8 (batch, 512-token q-chunk) units are processed in order;
after each PAIR of chunks completes, a small (256 KB) AllToAll immediately
redistributes that pair's head-sharded attention outputs as 128-token
full-channel slivers (core r gets sliver r of the pair).  Each core then
runs the output projection for its sliver as PE filler work between
attention entries — so all collectives and nearly all projection work are
hidden under later attention, and only the LAST pair's collective (~7us)
plus one sliver projection (~4us) sit on the critical tail.

Compute dtype: bf16 on TensorE with fp32 PSUM accumulation.

Kernel layout choices (carried over from the previous version):
- x is shipped pre-transposed and pre-tiled (xT [NQ, KC, 128, TQ] bf16) so
  the kqv projection produces K^T/Q^T/V^T directly ([dim, T], dims on
  partitions) and can start as soon as the first column-chunk lands.
- kqv bias is folded into the PSUM->SBUF evacuation (DVE tensor_scalar).
- Scores are computed transposed, sT[k, q] = kT_blk.T @ qT, so softmax's
  denominator folds into the AV matmul as an extra ones-column of V
  (lhsT = [v | ones] -> row 64 of yT accumulates sum_k exp).
- Consecutive unmasked score blocks are PAIRED into a 2-bank PSUM tile so
  each ACT exp call covers 1024 columns (halves the per-instruction ACT
  bubble; ACT is the bottleneck of the second half).
- Causal masking: upper-triangle-only score blocks are never computed; the
  diagonal blocks get a tril mask via gpsimd.affine_select.
- Division by the denominator: reciprocal_approx_fast (DVE) +
  partition_broadcast (GPSIMD) + tensor_mul (DVE).
- Sliver projection folds its bias via a ones-row (K=1 chunk).
- Emission interleaves batch 1's kqv with batch 0's attention to keep the
  TensorEngine dense; junk matmuls fill remaining PE gaps to hold the
  DVFS p-state up (notably during the final collective, so the exposed
  sliver projection runs at full clock).
"""

import hashlib
import numpy as np
import ml_dtypes

B, T, C, H = 2, 2048, 1024, 16
HD = C // H            # 64
NCORES = 8
TQ = 512               # q-chunk width
NJ = T // 128          # 16 k-blocks
NQ = T // TQ           # 4 q-chunks
KC = C // 128          # 8 contraction chunks
NCH = 8                # chunks = B * NQ
NG = NCH // 2          # collective groups (chunk pairs)
SLV = 128              # sliver width (tokens) per core per group

bfloat16 = ml_dtypes.bfloat16


def _chunk_bq(c):
    return c // NQ, c % NQ


# ---------------------------------------------------------------- schedules
def _make_schedule(att_mask):
    """Per q-chunk list of (j, n_off, n_len, masked).

    masked is None (no mask), 'tril' (apply causal tril to slab cols 0:128),
    or an int index into the general mask table.
    """
    m = np.asarray(att_mask).reshape(T, T)
    tril = np.tril(np.ones((T, T), m.dtype))
    if np.array_equal(m, tril):
        sched = []
        for Q in range(NQ):
            ent = [(j, 0, TQ, None) for j in range(4 * Q)]
            for j in range(4 * Q, 4 * Q + 4):
                n_off = 128 * (j - 4 * Q)
                ent.append((j, n_off, TQ - n_off, "tril"))
            sched.append(ent)
        return sched, None

    masks = []
    mask_ids = {}
    sched = []
    for Q in range(NQ):
        ent = []
        for j in range(NJ):
            blk = m[Q * TQ:(Q + 1) * TQ, j * 128:(j + 1) * 128].T  # [128k,512q]
            if not blk.any():
                continue
            if blk.all():
                ent.append((j, 0, TQ, None))
                continue
            key = blk.tobytes()
            if key not in mask_ids:
                mask_ids[key] = len(masks)
                masks.append(blk.astype(np.float32))
            ent.append((j, 0, TQ, mask_ids[key]))
        sched.append(ent)
    masks = np.stack(masks) if masks else None
    return sched, masks


def _group_entries(ents):
    """Group consecutive full-width unmasked entries into pairs."""
    groups = []
    i = 0
    while i < len(ents):
        j, n_off, n_len, mid = ents[i]
        if (mid is None and n_len == TQ and i + 1 < len(ents)
                and ents[i + 1][3] is None and ents[i + 1][2] == TQ):
            groups.append((ents[i], ents[i + 1]))
            i += 2
        else:
            groups.append((ents[i],))
            i += 1
    return groups


def _sched_key(sched, masks):
    h = hashlib.sha256(repr(sched).encode())
    if masks is not None:
        h.update(masks.tobytes())
    return h.hexdigest()


# ---------------------------------------------------------------- builder
_BUILD_CACHE = {}


def _build(sched, masks):
    from concourse import bacc, tile, mybir
    from concourse.masks import make_identity

    BF16, F32 = mybir.dt.bfloat16, mybir.dt.float32
    n_masks = 0 if masks is None else masks.shape[0]

    nc = bacc.Bacc("TRN2", target_bir_lowering=False, debug=False,
                   num_devices=NCORES)

    # -------- I/O ----------------------------------------------------------
    xT_d = [nc.dram_tensor(f"xT{b}", [NQ, KC, 128, TQ], BF16,
                           kind="ExternalInput") for b in range(B)]
    wk_d = nc.dram_tensor("wk", [C, 6 * HD], BF16, kind="ExternalInput")
    bk_d = nc.dram_tensor("bk", [128, 3], F32, kind="ExternalInput")
    wp_d = nc.dram_tensor("wp", [C, C], BF16, kind="ExternalInput")
    bp_d = nc.dram_tensor("bp", [1, C], BF16, kind="ExternalInput")
    if n_masks:
        mk_d = nc.dram_tensor("mk", [n_masks * 128, TQ], BF16,
                              kind="ExternalInput")
    out_d = nc.dram_tensor("out", [NG, SLV, C], BF16, kind="ExternalOutput")

    VW = 2 * HD + 2     # v_ext tile width: [vA | onesA | vB | onesB] = 130

    with tile.TileContext(nc) as tc:
        with tc.tile_pool(name="big", bufs=1) as big, \
             tc.tile_pool(name="work", bufs=1) as work, \
             tc.tile_pool(name="pmm", bufs=2, space="PSUM") as pmm, \
             tc.tile_pool(name="pqk", bufs=2, space="PSUM") as pqk, \
             tc.tile_pool(name="pyt", bufs=2, space="PSUM") as pyt, \
             tc.tile_pool(name="dram", bufs=1, space="DRAM") as dram:

            # ---- persistent SBUF tensors ----------------------------------
            wk = [big.tile([128, 6 * HD], BF16, name=f"wk{k}", tag=f"wk{k}")
                  for k in range(KC)]
            bkp = big.tile([128, 3], F32, name="bkp", tag="bkp")
            xT = [[big.tile([128, T], BF16, name=f"xT{b}_{k}", tag=f"xT{b}_{k}")
                   for k in range(KC)] for b in range(B)]
            ones_r = big.tile([1, T], BF16, name="ones_r", tag="ones_r")
            wp = [big.tile([128, C], BF16, name=f"wp{k}", tag=f"wp{k}")
                  for k in range(KC)]
            bp = big.tile([1, C], BF16, name="bp", tag="bp")
            ident = big.tile([128, 128], BF16, name="ident", tag="ident")

            # input DMAs: wk first, then xT0 (tiled), wp, xT1 — issued
            # round-robin across engines so descriptor generation (the
            # DIRECT2D writes on the issuing sequencer) parallelizes
            _eng = [nc.sync, nc.scalar, nc.gpsimd]
            _ei = [0]

            def _dma(out, in_):
                _eng[_ei[0] % len(_eng)].dma_start(out=out, in_=in_)
                _ei[0] += 1

            for k in range(KC):
                _dma(wk[k][:, :], wk_d.ap()[k * 128:(k + 1) * 128, :])
            _dma(bkp[:, :], bk_d.ap())
            for b in range(B):
                for n in range(NQ):
                    for k in range(KC):
                        _dma(xT[b][k][:, n * TQ:(n + 1) * TQ],
                             xT_d[b].ap()[n, k, :, :])
                if b == 0:
                    # wp/bp are needed only by the projection, but issue
                    # their DMAs early so they aren't queued behind later
                    # dependency-stalled DMAs
                    for k in range(KC):
                        _dma(wp[k][:, :], wp_d.ap()[k * 128:(k + 1) * 128, :])
                    _dma(bp[:, :], bp_d.ap())

            if n_masks:
                mks = big.tile([128, n_masks * TQ], BF16, name="mks",
                               tag="mks")
                for i in range(n_masks):
                    nc.sync.dma_start(out=mks[:, i * TQ:(i + 1) * TQ],
                                      in_=mk_d.ap()[i * 128:(i + 1) * 128, :])

            # per-batch attention tensors
            kT = [big.tile([128, T], BF16, name=f"kT{b}", tag=f"kT{b}")
                  for b in range(B)]
            qT = [big.tile([128, T], BF16, name=f"qT{b}", tag=f"qT{b}")
                  for b in range(B)]
            vT = [big.tile([128, T], BF16, name=f"vT{b}", tag=f"vT{b}")
                  for b in range(B)]
            vx = [big.tile([128, NJ * VW], BF16, name=f"vx{b}", tag=f"vx{b}")
                  for b in range(B)]
            yT = [big.tile([128, T], BF16, name=f"yT{b}", tag=f"yT{b}")
                  for b in range(B)]

            # collective buffers: one in/out pair per chunk-pair group
            a2a_in = [dram.tile([NCORES * 128, SLV], BF16,
                                name=f"a2a_in{g}", tag=f"a2a_in{g}")
                      for g in range(NG)]
            a2a_out = [dram.tile([NCORES * 128, SLV], BF16,
                                 name=f"a2a_out{g}", tag=f"a2a_out{g}")
                       for g in range(NG)]
            yg = [big.tile([128, C], BF16, name=f"yg{g}", tag=f"yg{g}")
                  for g in range(NG)]

            _tail = [nc.sync, nc.scalar]

            dst = {0: kT, 1: qT, 2: vT}

            def kqv_steps(b, ms=(0, 1, 2)):
                # kqvT[m-tile] = wk[:,m].T @ xT; bias folded into evacuation
                for m in ms:
                    for n in range(NQ):
                        ps = pmm.tile([128, TQ], F32, name="kqv_ps", tag="mm",
                                      bufs=2)
                        for k in range(KC):
                            nc.tensor.matmul(
                                ps[:, :],
                                wk[k][:, m * 128:(m + 1) * 128],
                                xT[b][k][:, n * TQ:(n + 1) * TQ],
                                start=(k == 0), stop=(k == KC - 1))
                        nc.vector.tensor_scalar_add(
                            dst[m][b][:, n * TQ:(n + 1) * TQ], ps[:, :],
                            bkp[:, m:m + 1])
                        yield

            def transpose_steps(b):
                vx_v = vx[b].rearrange("p (t c) -> p t c", t=NJ)
                for t in range(NJ):
                    tr = pmm.tile([128, 128], BF16, name="tr_ps", tag="mm",
                                  bufs=2)
                    nc.tensor.transpose(tr[:, :],
                                        vT[b][:, t * 128:(t + 1) * 128],
                                        ident[:, :])
                    o = vx_v[:, t, :].rearrange("p (u c) -> p u c", u=2)
                    nc.vector.tensor_copy(
                        o[:, :, 0:HD], tr.rearrange("p (u c) -> p u c", u=2))
                    if t % 4 == 3:
                        yield

            def stage_chunk(c):
                # stage the 4 slivers of chunk c into its group's a2a_in,
                # and fire the collective + yg gather after odd chunks
                b, Q = _chunk_bq(c)
                g, half = c // 2, c % 2
                for r4 in range(4):
                    r = 4 * half + r4
                    _tail[r4 % 2].dma_start(
                        out=a2a_in[g][r * 128:(r + 1) * 128, :],
                        in_=yT[b][:, Q * TQ + r4 * SLV:
                                  Q * TQ + (r4 + 1) * SLV])
                if half == 1:
                    nc.gpsimd.collective_compute(
                        "AllToAll", mybir.AluOpType.bypass,
                        replica_groups=[list(range(NCORES))],
                        ins=[a2a_in[g].opt()], outs=[a2a_out[g].opt()])
                    for k in range(KC):
                        _tail[k % 2].dma_start(
                            out=yg[g][:, k * 128:(k + 1) * 128],
                            in_=a2a_out[g][k * 128:(k + 1) * 128, :])

            def proj_unit(g, nch):
                # project sliver g, output-half nch: [128tok, 512out]
                ps = pmm.tile([128, TQ], F32, name="proj_ps", tag="mm",
                              bufs=2)
                nc.tensor.matmul(
                    ps[:, :], ones_r[0:1, 0:128],
                    bp[0:1, nch * TQ:(nch + 1) * TQ],
                    start=True, stop=False)
                for k in range(KC):
                    nc.tensor.matmul(
                        ps[:, :], yg[g][:, k * 128:(k + 1) * 128],
                        wp[k][:, nch * TQ:(nch + 1) * TQ],
                        start=False, stop=(k == KC - 1))
                osb = work.tile([128, TQ], BF16, name="osb", tag="osb",
                                bufs=3)
                nc.vector.tensor_copy(osb[:, :], ps[:, :])
                _tail[nch].dma_start(
                    out=out_d.ap()[g, :, nch * TQ:(nch + 1) * TQ],
                    in_=osb[:, :])

            jps = [None]

            def junk_mm():
                # filler matmul: keeps the PE activity monitor from
                # re-throttling the clock during ACT-bound stretches
                jp = pmm.tile([128, TQ], F32, name="junk_ps", tag="mm",
                              bufs=2)
                nc.tensor.matmul(jp[:, :], wk[0][:, 0:128],
                                 xT[1][0][:, 0:TQ], start=True, stop=True,
                                 skip_group_check=True)
                jps[0] = jp

            # fill queue: (ready_after_chunk, emit_fn); popped between
            # attention groups once the gate chunk is far enough behind
            fillq = []

            def pop_fill(cur_chunk, use_junk=True):
                if fillq and fillq[0][0] < cur_chunk:
                    fillq.pop(0)[1]()
                elif use_junk:
                    junk_mm()

            def attn_steps(c):
                b, Q = _chunk_bq(c)
                ents = sched[Q]
                if not ents:
                    for t in range(2):
                        nc.vector.memset(
                            yT[b][HD * t:HD * (t + 1), Q * TQ:(Q + 1) * TQ],
                            0.0)
                    stage_chunk(c)
                    return
                groups = _group_entries(ents)
                yps = [pyt.tile([HD + 1, TQ], F32, name=f"y_ps{t}", tag="yt",
                                bufs=2) for t in range(2)]
                n_av = {0: 0, 1: 0}   # AV matmuls emitted so far per head
                tot = sum(2 if (mid == "tril" and n_len > 128) else 1
                          for (j, n_off, n_len, mid) in ents)
                total_av = {0: tot, 1: tot}

                def emit_avs(avs):
                    # avs: list of (t, j, src, o_off, o_len)
                    for t, j, src, o_off, o_len in avs:
                        first = n_av[t] == 0
                        last = n_av[t] == total_av[t] - 1
                        nc.tensor.matmul(
                            yps[t][:, o_off:o_off + o_len],
                            vx[b][:, j * VW + t * (HD + 1):
                                  j * VW + (t + 1) * (HD + 1)],
                            src, start=first, stop=last,
                            skip_group_check=True)
                        n_av[t] += 1

                # Per group: both heads' QK matmuls back-to-back (they hit
                # disjoint PE row-groups and can run concurrently), then one
                # exp per head over the group's whole width, then the
                # previous group's AV matmuls (one-group lag so the TensorE
                # never waits on ACT).
                pending = []
                for grp in groups:
                    gw = sum(e[2] for e in grp)     # group column width
                    cur = []
                    sps = []
                    for t in range(2):
                        sp = pqk.tile([128, 2 * TQ], F32, name="s_ps",
                                      tag="qk", bufs=2)
                        o = 0
                        for (j, n_off, n_len, mid) in grp:
                            nc.tensor.matmul(
                                sp[:, o:o + n_len],
                                kT[b][HD * t:HD * (t + 1),
                                      j * 128:(j + 1) * 128],
                                qT[b][HD * t:HD * (t + 1),
                                      Q * TQ + n_off:(Q + 1) * TQ],
                                start=True, stop=True, skip_group_check=True)
                            o += n_len
                        sps.append(sp)
                    for t in range(2):
                        sp = sps[t]
                        slab = work.tile([128, 2 * TQ], BF16, name="slab",
                                         tag="slab", bufs=6)
                        nc.scalar.activation(
                            slab[:, 0:gw], sp[:, 0:gw],
                            mybir.ActivationFunctionType.Exp, scale=0.125)
                        o = 0
                        for (j, n_off, n_len, mid) in grp:
                            if mid == "tril":
                                slab2 = work.tile([128, 128], BF16,
                                                  name="slab2", tag="slab2",
                                                  bufs=6)
                                nc.gpsimd.affine_select(
                                    out=slab2[:, :], in_=slab[:, o:o + 128],
                                    compare_op=mybir.AluOpType.is_ge,
                                    fill=0.0, base=0, pattern=[[1, 128]],
                                    channel_multiplier=-1)
                                cur.append((t, j, slab2[:, :], n_off, 128))
                                if n_len > 128:
                                    cur.append((t, j,
                                                slab[:, o + 128:o + n_len],
                                                n_off + 128, n_len - 128))
                            elif mid is not None:
                                slab2 = work.tile([128, TQ], BF16,
                                                  name="slab2m", tag="slab2m",
                                                  bufs=4)
                                nc.vector.tensor_mul(
                                    slab2[:, 0:n_len], slab[:, o:o + n_len],
                                    mks[:, mid * TQ:mid * TQ + n_len])
                                cur.append((t, j, slab2[:, 0:n_len],
                                            n_off, n_len))
                            else:
                                cur.append((t, j, slab[:, o:o + n_len],
                                            n_off, n_len))
                            o += n_len
                    emit_avs(pending)
                    pending = cur
                    yield
                emit_avs(pending)
                # normalize: yT /= denominator (row HD of y psum)
                for t in range(2):
                    den0 = work.tile([1, TQ], F32, name="den0", tag="den0",
                                     bufs=4)
                    nc.vector.tensor_copy(den0[:, :], yps[t][HD:HD + 1, :])
                    den = work.tile([1, TQ], F32, name="den", tag="den",
                                    bufs=4)
                    nc.vector.reciprocal_approx_fast(den[:, :], den0[:, :])
                    bc = work.tile([HD, TQ], F32, name="bc", tag="bc", bufs=4)
                    nc.gpsimd.partition_broadcast(bc[:, :], den[:, :])
                    nc.vector.tensor_mul(
                        yT[b][HD * t:HD * (t + 1), Q * TQ:(Q + 1) * TQ],
                        yps[t][0:HD, :], bc[:, :])
                stage_chunk(c)

            # ---- interleaved emission -------------------------------------
            for _ in kqv_steps(0, (0, 1)):
                pass
            make_identity(nc, ident[:, :])
            nc.vector.memset(ones_r[:, :], 1.0)
            for b in range(B):
                vx_v = vx[b].rearrange("p (t c) -> p t c", t=NJ)
                nc.vector.memset(vx_v[:, :, HD::HD + 1], 1.0)

            # attention b0 starts on QK/exp as soon as kT/qT exist; the v
            # projection + transposes stream in 1:1 with the first attention
            # groups.  Emission order guarantees each vx block's writer
            # precedes its first AV reader (Tile deps follow program order).
            def v_stream(b):
                ts = transpose_steps(b)   # yields after blocks 4n..4n+3
                kv = kqv_steps(b, (2,))   # yields per v n-tile
                for n in range(NQ):
                    next(kv)
                    next(ts)
                    yield

            s1 = attn_steps(0)            # chunk 0 (b0 Q0): 4 groups
            s0 = v_stream(0)
            for _ in s0:
                next(s1, None)
            for _ in s1:
                pass

            # chunks 1-3 (b0 Q1-Q3) round-robin with kqv/transposes b1;
            # proj slivers for g0 feed in as PE filler once ready
            def chain_steps(*gens):
                for g_ in gens:
                    yield from g_

            s1 = chain_steps(*[attn_steps(c) for c in range(1, 4)])
            s2 = chain_steps(kqv_steps(1), transpose_steps(1))
            fillq.append((2, lambda: proj_unit(0, 0)))
            fillq.append((2, lambda: proj_unit(0, 1)))
            s2_live = True

            # interleave: 3 s1 groups then 2 s2 steps (24 groups vs 16 steps)
            n1 = 0
            while True:
                if next(s1, StopIteration) is StopIteration:
                    break
                n1 += 1
                cur_c = 1 + (0 if n1 <= 6 else (1 if n1 <= 14 else 2))
                if n1 % 3 == 0 and s2_live:
                    s2_live = next(s2, StopIteration) is not StopIteration
                    if s2_live:
                        s2_live = next(s2, StopIteration) is not StopIteration
                if n1 % 4 == 2:
                    pop_fill(cur_c, use_junk=False)
            while s2_live:
                s2_live = next(s2, StopIteration) is not StopIteration

            # chunks 4-7 (b1): ACT-bound; fill PE with proj slivers + junk
            fillq.append((4, lambda: proj_unit(1, 0)))
            fillq.append((4, lambda: proj_unit(1, 1)))
            fillq.append((6, lambda: proj_unit(2, 0)))
            fillq.append((6, lambda: proj_unit(2, 1)))
            for c in range(4, NCH):
                for _ in attn_steps(c):
                    pop_fill(c)

            # tail: keep PE warm while collective g3 flies, then project
            for _ in range(16):
                junk_mm()
            if jps[0] is not None:
                josb = work.tile([128, TQ], BF16, name="josb", tag="osb",
                                 bufs=3)
                nc.vector.tensor_copy(josb[:, :], jps[0][:, :])
                nc.scalar.dma_start(out=out_d.ap()[NG - 1, :, 0:TQ],
                                    in_=josb[:, :])
            proj_unit(3, 0)
            proj_unit(3, 1)

    nc.compile()
    return nc


# ---------------------------------------------------------------- host glue
def _prep_in_maps(x, att_mask, w_kqv, b_kqv, w_proj, b_proj, masks):
    bf = bfloat16
    xTt = np.empty((B, NQ, KC, 128, TQ), dtype=bf)
    for b in range(B):
        xt = np.ascontiguousarray(x[b].T.astype(bf))          # [C, T]
        xTt[b] = xt.reshape(KC, 128, NQ, TQ).transpose(2, 0, 1, 3)

    wk3 = w_kqv.reshape(C, H, 3, HD)
    bk3 = b_kqv.reshape(H, 3, HD)
    in_maps = []
    for core in range(NCORES):
        hA, hB = 2 * core, 2 * core + 1
        wk_c = np.concatenate(
            [np.concatenate([wk3[:, hA, s, :], wk3[:, hB, s, :]], axis=1)
             for s in range(3)], axis=1).astype(bf)           # [1024, 384]
        bk_c = np.stack(
            [np.concatenate([bk3[hA, s], bk3[hB, s]])
             for s in range(3)], axis=1).astype(np.float32)   # [128, 3]
        im = {
            "xT0": xTt[0], "xT1": xTt[1],
            "wk": np.ascontiguousarray(wk_c),
            "bk": np.ascontiguousarray(bk_c),
            "wp": w_proj.astype(bf),
            "bp": b_proj.reshape(1, C).astype(bf),
        }
        if masks is not None:
            im["mk"] = masks.astype(bf).reshape(-1, TQ)
        in_maps.append(im)
    return in_maps


def kernel(x, att_mask, w_kqv, b_kqv, w_proj, b_proj, n_head):
    from concourse.bass_utils import run_bass_kernel_spmd

    x = np.asarray(x, dtype=np.float32)
    att_mask = np.asarray(att_mask)
    w_kqv = np.asarray(w_kqv, dtype=np.float32)
    b_kqv = np.asarray(b_kqv, dtype=np.float32)
    w_proj = np.asarray(w_proj, dtype=np.float32)
    b_proj = np.asarray(b_proj, dtype=np.float32)
    n_head = int(n_head)
    assert x.shape == (B, T, C) and n_head == H

    sched, masks = _make_schedule(att_mask)
    key = _sched_key(sched, masks)
    if key not in _BUILD_CACHE:
        _BUILD_CACHE[key] = _build(sched, masks)
    nc = _BUILD_CACHE[key]

    in_maps = _prep_in_maps(x, att_mask, w_kqv, b_kqv, w_proj, b_proj, masks)
    res = run_bass_kernel_spmd(nc, in_maps, core_ids=list(range(NCORES)))

    out = np.empty((B, T, C), dtype=np.float32)
    for core in range(NCORES):
        arr = res.results[core]["out"].astype(np.float32)   # [NG, SLV, C]
        for g in range(NG):
            c = 2 * g + (1 if core >= 4 else 0)
            b, Q = _chunk_bq(c)
            off = Q * TQ + (core % 4) * SLV
            out[b, off:off + SLV, :] = arr[g]
    return out
